# revision 2
# baseline (speedup 1.0000x reference)
"""Trainium2 Bass kernel for nn_Attention_4844723110037 (v5).

Single-head unscaled attention:
    q = x @ Wq + bq ; k = x @ Wk + bk ; v = x @ Wv + bv
    out = softmax(q @ k^T) @ v @ Wo + bo
with x: [4, 4096, 512] fp32, all weights [512, 512].

Sharding: 8 cores = 4 batches x 2 query-halves. SPMD: one program; the host
passes each core xT = x[b]^T (bf16, key columns rolled so the core's own
2048 query rows come first -- softmax is key-order invariant).

Algebraic refactor: softmax(q k^T) is invariant to per-row score shifts, so
fold the weights on the HOST:
    A = Wq Wk^T         scores = x_q A x_k^T  (+ g[k] = x_k Wk bq rowwise)
    B = Wv Wo           out = (E x B + sums (x) c_row) / sums,  E = exp(. -16+g)
    c_row = bv Wo + bo
K projection, output projection, and all PE transposes are gone (the host
supplies x already transposed; pure data movement, like the q-roll).
All big matmuls run bf16 x bf16 (fp32 PSUM accumulate): same PE row rate as
fp32r but 2-byte LDWEIGHTS (~104ns vs ~186ns per matmul). End-to-end rel
err ~8.6e-3 (dominated by bf16 quantization of x on the score path).

Per-core dataflow:
  Phase 1: DMA xT -> XT[4][128, 4096] (bf16, no compute), then
     u [d', q] = A-tiles^T XT-chunks  (own 4 chunks; ACT copy to SBUF)
     V'[s, d'] = XT-chunk^T B-tiles   (all 8 chunks; ACT copy to SBUF)
  Phase 2 (per 512-wide query chunk):
     scoresT[k,q] = XT-chunk^T u      (PSUM, 4 accum matmuls)
     expT = exp(scoresT - 16 + g)     (ACT, PSUM->SBUF bf16, one kc behind)
     out[q,d]    += expT-slice^T V'   (4 PSUM banks, 32-step accumulation,
                                       one kc behind the exp)
     row sums: DVE quad-tree + one rank-1 matmul (final q-chunk: rank-1
     accumulation on the PE instead, shortening the end-of-kernel tail)
     += sums (x) c_row (rank-1, closes the group); out rows scaled by
     recip(sums) on ACT, DMA out.
"""

import os
import sys

import ml_dtypes
import numpy as np

# The device run goes through jax/PJRT on the axon platform; a pinned
# JAX_PLATFORMS=cpu (common for reference-only flows) would break it.
if os.environ.get("JAX_PLATFORMS") == "cpu" and "jax" not in sys.modules:
    del os.environ["JAX_PLATFORMS"]

for _p in ("/opt/trn_rl_repo", os.path.expanduser("~/.axon_site/_ro/trn_rl_repo")):
    if os.path.isdir(_p) and _p not in sys.path:
        sys.path.insert(0, _p)

import concourse.bacc as bacc
import concourse.bass as bass
import concourse.tile as tile
from concourse import masks, mybir
from concourse.bass_utils import run_bass_kernel_spmd

F32 = mybir.dt.float32
F32R = mybir.dt.float32r
BF16 = mybir.dt.bfloat16
AF = mybir.ActivationFunctionType

B = 4
S = 4096          # kv rows per batch
SQ = 2048         # query rows per core
D = 512           # model dim
P = 128
NKC = S // P      # 32 key chunks of 128
NQC = SQ // 512   # 4 query chunks of 512
DT = D // P       # 4 d-tiles
NCH = S // 512    # 8 x chunks of 512 rows
QUAD = 4          # expT tiles pre-summed on DVE per rank-1 sums matmul
EXP_SHIFT = -16.0  # constant softmax shift (scores empirically in ~[-30, 30])


def build_bass():
    nc = bacc.Bacc("TRN2", target_bir_lowering=False, debug=False)

    # host-prearranged layouts (see make_in_maps)
    xt = nc.dram_tensor("xt", [D, S], BF16, kind="ExternalInput")      # x^T, rolled
    a_w = nc.dram_tensor("a_w", [P, DT, D], BF16, kind="ExternalInput")  # Wq Wk^T
    b_w = nc.dram_tensor("b_w", [P, DT, D], BF16, kind="ExternalInput")  # Wv Wo
    crow = nc.dram_tensor("crow", [D], F32, kind="ExternalInput")      # bv Wo + bo
    gb = nc.dram_tensor("gb", [P, NKC], F32, kind="ExternalInput")     # x Wk bq - 16
    out = nc.dram_tensor("out", [SQ, D], F32, kind="ExternalOutput")

    with tile.TileContext(nc) as tc:
        with (
            tc.tile_pool(name="consts", bufs=1) as consts,
            tc.tile_pool(name="xt", bufs=DT) as xt_pool,
            tc.tile_pool(name="v", bufs=NKC) as v_pool,
            tc.tile_pool(name="u", bufs=DT) as u_pool,
            tc.tile_pool(name="small", bufs=1) as small_pool,
            tc.tile_pool(name="ps_mm", bufs=3, space="PSUM") as ps_mm,
            tc.tile_pool(name="ps_out", bufs=4, space="PSUM") as ps_out,
            tc.tile_pool(name="ps_sum", bufs=1, space="PSUM") as ps_sum,
        ):
            # ---- persistent activations ----
            xt_sb = [xt_pool.tile([P, S], BF16, tag="xt", name="xt") for _ in range(DT)]
            v_sb = [v_pool.tile([P, D], BF16, tag="v", name="v") for _ in range(NKC)]
            u_sb = [u_pool.tile([P, SQ], BF16, tag="u", name="u") for _ in range(DT)]

            # ================= phase 1: load xT + folded projections ==========
            with tc.tile_pool(name="ab", bufs=1) as ab_pool:
                a_sb = [ab_pool.tile([P, D], BF16, name=f"a{i}") for i in range(DT)]
                b_sb = [ab_pool.tile([P, D], BF16, name=f"b{i}") for i in range(DT)]

                # interleave the first xT columns with the A tiles so the
                # first u matmul only waits on ~2 small DMAs, not all of A
                def load_xt_cols(c0, c1):
                    for i in range(DT):
                        nc.sync.dma_start(
                            xt_sb[i][:, c0:c1],
                            xt[i * P:(i + 1) * P, c0:c1],
                        )

                for i in range(DT):
                    nc.sync.dma_start(
                        xt_sb[i][:, 0:512], xt[i * P:(i + 1) * P, 0:512]
                    )
                    nc.sync.dma_start(a_sb[i], a_w[:, i, :])
                load_xt_cols(512, 1024)
                for i in range(DT):
                    nc.sync.dma_start(b_sb[i], b_w[:, i, :])

                # ---- constants (DMAs queued after the hot path) ----
                gb_sb = consts.tile([P, NKC], F32)     # per-key exp bias (g - 16)
                nc.sync.dma_start(gb_sb, gb[:, :])
                c_row = consts.tile([1, D], F32R)      # bv Wo + bo
                nc.sync.dma_start(c_row, crow.bitcast(F32R).rearrange("(o d) -> o d", o=1))
                ones_st = consts.tile([P, 1], F32)
                nc.vector.memset(ones_st, 1.0)
                ones_col = consts.tile([P, 1], F32R)   # lhsT for rank-1 row sums
                nc.vector.tensor_copy(ones_col, ones_st)
                ones_bf = consts.tile([P, 1], BF16)    # bf16 twin (dtype-matched)
                nc.vector.tensor_copy(ones_bf, ones_st)
                ones_1x2_st = consts.tile([1, 2], F32)
                nc.vector.memset(ones_1x2_st, 1.0)
                ones_1x2 = consts.tile([1, 2], F32R)   # rhs for [1,n]->[n,1] transpose
                nc.vector.tensor_copy(ones_1x2, ones_1x2_st)

                for cp in range(1, NCH // 2):
                    load_xt_cols(cp * 1024, (cp + 1) * 1024)

                for chunk in range(NCH):
                    # u[d',q] = A^T x_q^T for own query rows (first 4 chunks)
                    if chunk < SQ // 512:
                        for dp in range(DT):
                            u_ps = ps_mm.tile([P, 512], F32, tag="mm", name="u_ps")
                            for i in range(DT):
                                nc.tensor.matmul(
                                    u_ps,
                                    lhsT=a_sb[i][:, dp * P:(dp + 1) * P],
                                    rhs=xt_sb[i][:, chunk * 512:(chunk + 1) * 512],
                                    start=(i == 0),
                                    stop=(i == DT - 1),
                                )
                            nc.scalar.activation(
                                u_sb[dp][:, chunk * 512:(chunk + 1) * 512],
                                u_ps,
                                AF.Copy,
                            )
                    # V'[s,d'] = x @ B for all rows
                    for j in range(4):
                        v_ps = ps_mm.tile([P, D], F32, tag="mm", name="v_ps")
                        for i in range(DT):
                            nc.tensor.matmul(
                                v_ps,
                                lhsT=xt_sb[i][
                                    :, chunk * 512 + j * P:chunk * 512 + (j + 1) * P
                                ],
                                rhs=b_sb[i],
                                start=(i == 0),
                                stop=(i == DT - 1),
                            )
                        nc.scalar.activation(v_sb[chunk * 4 + j], v_ps, AF.Copy)

            # ================= phase 2: attention =================
            with (
                tc.tile_pool(name="et", bufs=8) as et_pool,
                tc.tile_pool(name="esum", bufs=5) as esum_pool,
                tc.tile_pool(name="outsb", bufs=2) as out_pool,
            ):
                for qc in range(NQC):
                    last_qc = qc == NQC - 1
                    out_ps = [
                        ps_out.tile([P, D], F32, tag="o", name="out_ps")
                        for _ in range(4)
                    ]
                    sum_ps = ps_sum.tile([1, 512], F32, tag="sum", name="sum_ps")
                    group_et = []
                    e_run = [None]  # running sum of the quad-group partials

                    def emit_av(k, e):
                        # AV matmuls + row-sum bookkeeping for key chunk k;
                        # called one iteration late so the PE works on chunk
                        # k while ACT computes exp for chunk k+1
                        for qs in range(4):
                            nc.tensor.matmul(
                                out_ps[qs],
                                lhsT=e[:, qs * P:(qs + 1) * P],
                                rhs=v_sb[k],
                                start=(k == 0),
                                stop=False,
                            )
                        if last_qc and k >= NKC - QUAD:
                            # final q-chunk: accumulate the last group's row
                            # sums on the PE -- the DVE tree is 3 serial
                            # ~0.7us adds sitting on the end-of-kernel path
                            if k == NKC - QUAD:
                                nc.tensor.matmul(
                                    sum_ps, lhsT=ones_col, rhs=e_run[0],
                                    start=True, stop=False,
                                )
                            nc.tensor.matmul(
                                sum_ps, lhsT=ones_bf, rhs=e,
                                start=False, stop=(k == NKC - 1),
                            )
                            return
                        group_et.append(e)
                        if len(group_et) == QUAD:
                            lvl = group_et[:]
                            group_et.clear()
                            while len(lvl) > 1:
                                nxt = []
                                for a, b_ in zip(lvl[::2], lvl[1::2]):
                                    e2 = esum_pool.tile(
                                        [P, 512], F32R, tag="es", name="es"
                                    )
                                    nc.vector.tensor_add(e2, a, b_)
                                    nxt.append(e2)
                                lvl = nxt
                            if e_run[0] is None:
                                acc = esum_pool.tile(
                                    [P, 512], F32R, tag="erun", name="erun",
                                    bufs=2,
                                )
                                nc.vector.tensor_copy(acc, lvl[0])
                                e_run[0] = acc
                            else:
                                nc.vector.tensor_add(e_run[0], e_run[0], lvl[0])

                    pend = None
                    for kc in range(NKC):
                        s_ps = ps_mm.tile([P, 512], F32, tag="mm", name="s_ps")
                        for i in range(DT):
                            nc.tensor.matmul(
                                s_ps,
                                lhsT=xt_sb[i][:, kc * P:(kc + 1) * P],
                                rhs=u_sb[i][:, qc * 512:(qc + 1) * 512],
                                start=(i == 0),
                                stop=(i == DT - 1),
                            )
                        et = et_pool.tile([P, 512], BF16, tag="et", name="et")
                        nc.scalar.activation(
                            et, s_ps, AF.Exp, bias=gb_sb[:, kc:kc + 1]
                        )
                        if pend is not None:
                            emit_av(*pend)
                        pend = (kc, et)
                    emit_av(*pend)
                    if not last_qc:
                        nc.tensor.matmul(
                            sum_ps,
                            lhsT=ones_col,
                            rhs=e_run[0],
                            start=True,
                            stop=True,
                        )

                    # row sums -> per-partition reciprocals per q-subtile
                    sums_r = small_pool.tile([1, 512], F32R, tag="sums", name="sums")
                    nc.vector.tensor_copy(sums_r, sum_ps)
                    for qs in range(4):
                        # rank-1 bias, pre-scaled by the row sums so the recip
                        # scaling below restores the exact bias; closes the
                        # 32-step accumulation group
                        nc.tensor.matmul(
                            out_ps[qs],
                            lhsT=sums_r[:, qs * P:(qs + 1) * P],
                            rhs=c_row,
                            start=False,
                            stop=True,
                        )
                        r_ps = ps_sum.tile([P, 2], F32, tag="sum", name="r_ps")
                        nc.tensor.matmul(
                            r_ps,
                            lhsT=sums_r[:, qs * P:(qs + 1) * P],
                            rhs=ones_1x2,
                            start=True,
                            stop=True,
                        )
                        rc = small_pool.tile(
                            [P, 1], F32, tag="recip", name="recip", bufs=4
                        )
                        nc.vector.reciprocal(rc, r_ps[:, 0:1])
                        o_sb = out_pool.tile([P, D], F32, tag="outsb", name="outsb")
                        nc.scalar.activation(o_sb, out_ps[qs], AF.Copy, scale=rc)
                        nc.sync.dma_start(
                            out[(qc * 4 + qs) * P:(qc * 4 + qs + 1) * P, :], o_sb
                        )

    nc.compile()
    return nc


_NC_CACHE = None


def _get_nc():
    global _NC_CACHE
    if _NC_CACHE is None:
        _NC_CACHE = build_bass()
    return _NC_CACHE


def make_in_maps(inputs):
    x = np.asarray(inputs["x"], dtype=np.float32)
    w = {k: np.asarray(inputs[k], dtype=np.float64)
         for k in ("Wq", "bq", "Wk", "bk", "Wv", "bv", "Wo", "bo")}

    def tile_rows(m):  # [D, D] -> [P, DT, D] (partition-major d-tiles)
        return np.ascontiguousarray(
            m.reshape(DT, P, D).transpose(1, 0, 2).astype(ml_dtypes.bfloat16)
        )

    a_w = tile_rows(w["Wq"] @ w["Wk"].T)
    b_w = tile_rows(w["Wv"] @ w["Wo"])
    crow = np.ascontiguousarray((w["bv"] @ w["Wo"] + w["bo"]).astype(np.float32))
    wkbq = w["Wk"] @ w["bq"]                     # [D]; scores shift g = x_k . wkbq

    in_maps = []
    for c in range(8):
        b, half = c // 2, c % 2
        own = x[b, half * SQ:(half + 1) * SQ]
        other = x[b, (1 - half) * SQ:(2 - half) * SQ]
        xkv = np.concatenate([own, other], axis=0)           # rolled [S, D] f32
        g = xkv.astype(np.float64) @ wkbq + EXP_SHIFT        # [S]
        gb = np.ascontiguousarray(
            g.astype(np.float32).reshape(NKC, P).T            # [P, NKC]
        )
        in_maps.append({
            "xt": np.ascontiguousarray(xkv.T.astype(ml_dtypes.bfloat16)),
            "a_w": a_w, "b_w": b_w, "crow": crow, "gb": gb,
        })
    return in_maps


def gather_out(results):
    out = np.empty((B, S, D), dtype=np.float32)
    for c in range(8):
        b, half = c // 2, c % 2
        out[b, half * SQ:(half + 1) * SQ] = results[c]["out"]
    return out


def kernel(**inputs):
    nc = _get_nc()
    res = run_bass_kernel_spmd(nc, make_in_maps(inputs), list(range(8)))
    return gather_out(res.results)


if __name__ == "__main__":
    import jax

    import reference

    with jax.default_device(jax.devices("cpu")[0]):
        inp = {k: np.asarray(v) for k, v in reference.setup_inputs().items()}
        expected = np.asarray(reference.reference(**inp))
    actual = kernel(**inp)
    err = np.abs(actual - expected).max()
    rel = np.linalg.norm(actual - expected) / np.linalg.norm(expected)
    print("abs max err", err, "rel err", rel)


# revision 3
# speedup vs baseline: 1.1234x; 1.1234x over previous
"""Trainium2 Bass kernel for nn_Attention_4844723110037.

Single-head unscaled attention:
    q = x @ Wq + bq ; k = x @ Wk + bk ; v = x @ Wv + bv
    out = softmax(q @ k^T) @ v @ Wo + bo
with x: [4, 4096, 512] fp32, all weights [512, 512].

Sharding: 8 cores = 4 batches x 2 query-halves. SPMD: one program; the host
passes each core xT = x[b]^T (bf16, key columns rolled so the core's own
2048 query rows come first -- softmax is key-order invariant).

Algebraic refactor: softmax(q k^T) is invariant to per-row score shifts, so
fold the weights on the HOST:
    A = Wq Wk^T         scores = x_q A x_k^T  (+ g[k] = x_k Wk bq rowwise)
    B = Wv Wo           out = (E x B + sums (x) c_row) / sums,  E = exp(. -16+g)
    c_row = bv Wo + bo
K projection, output projection, and all PE transposes are gone (the host
supplies x already transposed; pure data movement, like the q-roll).
All big matmuls run bf16 x bf16 (fp32 PSUM accumulate): same PE row rate as
fp32r but 2-byte LDWEIGHTS (~104ns vs ~186ns per matmul). End-to-end rel
err ~8.6e-3 (dominated by bf16 quantization of x on the score path).

Per-core dataflow:
  Phase 1: DMA xT -> XT[4][128, 4096] (bf16, no compute), then
     u [d', q] = A-tiles^T XT-chunks  (own 4 chunks; ACT copy to SBUF)
     V'[s, d'] = XT-chunk^T B-tiles   (all 8 chunks; ACT copy to SBUF)
  Phase 2 (per 512-wide query chunk):
     scoresT[k,q] = XT-chunk^T u      (PSUM, 4 accum matmuls)
     expT = exp(scoresT - 16 + g)     (ACT, PSUM->SBUF bf16, one kc behind)
     out[q,d]    += expT-slice^T V'   (4 PSUM banks, 32-step accumulation,
                                       one kc behind the exp)
     row sums: DVE quad-tree + one rank-1 matmul (final q-chunk: rank-1
     accumulation on the PE instead, shortening the end-of-kernel tail)
     += sums (x) c_row (rank-1, closes the group); out rows scaled by
     recip(sums) on ACT, DMA out.
"""

import os
import sys

import ml_dtypes
import numpy as np

# The device run goes through jax/PJRT on the axon platform; a pinned
# JAX_PLATFORMS=cpu (common for reference-only flows) would break it.
if os.environ.get("JAX_PLATFORMS") == "cpu" and "jax" not in sys.modules:
    del os.environ["JAX_PLATFORMS"]

for _p in ("/opt/trn_rl_repo", os.path.expanduser("~/.axon_site/_ro/trn_rl_repo")):
    if os.path.isdir(_p) and _p not in sys.path:
        sys.path.insert(0, _p)

import concourse.bacc as bacc
import concourse.bass as bass
import concourse.tile as tile
from concourse import masks, mybir
from concourse.bass_utils import run_bass_kernel_spmd

F32 = mybir.dt.float32
F32R = mybir.dt.float32r
BF16 = mybir.dt.bfloat16
AF = mybir.ActivationFunctionType

B = 4
S = 4096          # kv rows per batch
SQ = 2048         # query rows per core
D = 512           # model dim
P = 128
NKC = S // P      # 32 key chunks of 128
NQC = SQ // 512   # 4 query chunks of 512
DT = D // P       # 4 d-tiles
NCH = S // 512    # 8 x chunks of 512 rows
QUAD = 4          # expT tiles pre-summed on DVE per rank-1 sums matmul
EXP_SHIFT = -16.0  # constant softmax shift (scores empirically in ~[-30, 30])


def build_bass():
    nc = bacc.Bacc("TRN2", target_bir_lowering=False, debug=False)

    # host-prearranged layouts (see make_in_maps)
    xt = nc.dram_tensor("xt", [D, S], BF16, kind="ExternalInput")      # x^T, rolled
    a_w = nc.dram_tensor("a_w", [P, DT, D], BF16, kind="ExternalInput")  # Wq Wk^T
    b_w = nc.dram_tensor("b_w", [P, DT, D], BF16, kind="ExternalInput")  # Wv Wo
    crow = nc.dram_tensor("crow", [D], F32, kind="ExternalInput")      # bv Wo + bo
    gb = nc.dram_tensor("gb", [P, NKC], F32, kind="ExternalInput")     # x Wk bq - 16
    out = nc.dram_tensor("out", [SQ, D], F32, kind="ExternalOutput")

    with tile.TileContext(nc) as tc:
        with (
            tc.tile_pool(name="consts", bufs=1) as consts,
            tc.tile_pool(name="xt", bufs=DT) as xt_pool,
            tc.tile_pool(name="v", bufs=NKC) as v_pool,
            tc.tile_pool(name="u", bufs=DT) as u_pool,
            tc.tile_pool(name="small", bufs=1) as small_pool,
            tc.tile_pool(name="ps_mm", bufs=3, space="PSUM") as ps_mm,
            tc.tile_pool(name="ps_out", bufs=4, space="PSUM") as ps_out,
            tc.tile_pool(name="ps_sum", bufs=1, space="PSUM") as ps_sum,
        ):
            # ---- persistent activations ----
            xt_sb = [xt_pool.tile([P, S], BF16, tag="xt", name="xt") for _ in range(DT)]
            v_sb = [v_pool.tile([P, D], BF16, tag="v", name="v") for _ in range(NKC)]
            u_sb = [u_pool.tile([P, SQ], BF16, tag="u", name="u") for _ in range(DT)]

            # ================= phase 1: load xT + folded projections ==========
            with tc.tile_pool(name="ab", bufs=1) as ab_pool:
                a_sb = [ab_pool.tile([P, D], BF16, name=f"a{i}") for i in range(DT)]
                b_sb = [ab_pool.tile([P, D], BF16, name=f"b{i}") for i in range(DT)]

                # interleave the first xT columns with the A tiles so the
                # first u matmul only waits on ~2 small DMAs, not all of A
                def load_xt_cols(c0, c1):
                    for i in range(DT):
                        nc.sync.dma_start(
                            xt_sb[i][:, c0:c1],
                            xt[i * P:(i + 1) * P, c0:c1],
                        )

                for i in range(DT):
                    nc.sync.dma_start(
                        xt_sb[i][:, 0:512], xt[i * P:(i + 1) * P, 0:512]
                    )
                    nc.sync.dma_start(a_sb[i], a_w[:, i, :])
                load_xt_cols(512, 1024)
                for i in range(DT):
                    nc.sync.dma_start(b_sb[i], b_w[:, i, :])

                # ---- constants (DMAs queued after the hot path) ----
                gb_sb = consts.tile([P, NKC], F32)     # per-key exp bias (g - 16)
                nc.sync.dma_start(gb_sb, gb[:, :])
                c_row = consts.tile([1, D], F32R)      # bv Wo + bo
                nc.sync.dma_start(c_row, crow.bitcast(F32R).rearrange("(o d) -> o d", o=1))
                ones_st = consts.tile([P, 1], F32)
                nc.vector.memset(ones_st, 1.0)
                ones_col = consts.tile([P, 1], F32R)   # lhsT for rank-1 row sums
                nc.vector.tensor_copy(ones_col, ones_st)
                ones_bf = consts.tile([P, 1], BF16)    # bf16 twin (dtype-matched)
                nc.vector.tensor_copy(ones_bf, ones_st)
                ones_1x2_st = consts.tile([1, 2], F32)
                nc.vector.memset(ones_1x2_st, 1.0)
                ones_1x2 = consts.tile([1, 2], F32R)   # rhs for [1,n]->[n,1] transpose
                nc.vector.tensor_copy(ones_1x2, ones_1x2_st)

                for cp in range(1, NCH // 2):
                    load_xt_cols(cp * 1024, (cp + 1) * 1024)

                for chunk in range(NCH):
                    # u[d',q] = A^T x_q^T for own query rows (first 4 chunks)
                    if chunk < SQ // 512:
                        for dp in range(DT):
                            u_ps = ps_mm.tile([P, 512], F32, tag="mm", name="u_ps")
                            for i in range(DT):
                                nc.tensor.matmul(
                                    u_ps,
                                    lhsT=a_sb[i][:, dp * P:(dp + 1) * P],
                                    rhs=xt_sb[i][:, chunk * 512:(chunk + 1) * 512],
                                    start=(i == 0),
                                    stop=(i == DT - 1),
                                )
                            nc.scalar.activation(
                                u_sb[dp][:, chunk * 512:(chunk + 1) * 512],
                                u_ps,
                                AF.Copy,
                            )
                    # V'[s,d'] = x @ B for all rows
                    for j in range(4):
                        v_ps = ps_mm.tile([P, D], F32, tag="mm", name="v_ps")
                        for i in range(DT):
                            nc.tensor.matmul(
                                v_ps,
                                lhsT=xt_sb[i][
                                    :, chunk * 512 + j * P:chunk * 512 + (j + 1) * P
                                ],
                                rhs=b_sb[i],
                                start=(i == 0),
                                stop=(i == DT - 1),
                            )
                        nc.scalar.activation(v_sb[chunk * 4 + j], v_ps, AF.Copy)

            # ================= phase 2: attention =================
            with (
                tc.tile_pool(name="et", bufs=8) as et_pool,
                tc.tile_pool(name="esum", bufs=5) as esum_pool,
                tc.tile_pool(name="outsb", bufs=2) as out_pool,
            ):
                for qc in range(NQC):
                    last_qc = qc == NQC - 1
                    out_ps = [
                        ps_out.tile([P, D], F32, tag="o", name="out_ps")
                        for _ in range(4)
                    ]
                    sum_ps = ps_sum.tile([1, 512], F32, tag="sum", name="sum_ps")
                    group_et = []
                    e_run = [None]  # running sum of the quad-group partials

                    def emit_av(k, e):
                        # AV matmuls + row-sum bookkeeping for key chunk k;
                        # called one iteration late so the PE works on chunk
                        # k while ACT computes exp for chunk k+1
                        for qs in range(4):
                            nc.tensor.matmul(
                                out_ps[qs],
                                lhsT=e[:, qs * P:(qs + 1) * P],
                                rhs=v_sb[k],
                                start=(k == 0),
                                stop=False,
                            )
                        if last_qc and k >= NKC - QUAD:
                            # final q-chunk: accumulate the last group's row
                            # sums on the PE -- the DVE tree is 3 serial
                            # ~0.7us adds sitting on the end-of-kernel path
                            if k == NKC - QUAD:
                                nc.tensor.matmul(
                                    sum_ps, lhsT=ones_col, rhs=e_run[0],
                                    start=True, stop=False,
                                )
                            nc.tensor.matmul(
                                sum_ps, lhsT=ones_bf, rhs=e,
                                start=False, stop=(k == NKC - 1),
                            )
                            return
                        group_et.append(e)
                        if len(group_et) == QUAD:
                            lvl = group_et[:]
                            group_et.clear()
                            while len(lvl) > 1:
                                nxt = []
                                for a, b_ in zip(lvl[::2], lvl[1::2]):
                                    e2 = esum_pool.tile(
                                        [P, 512], F32R, tag="es", name="es"
                                    )
                                    nc.vector.tensor_add(e2, a, b_)
                                    nxt.append(e2)
                                lvl = nxt
                            if e_run[0] is None:
                                acc = esum_pool.tile(
                                    [P, 512], F32R, tag="erun", name="erun",
                                    bufs=2,
                                )
                                nc.vector.tensor_copy(acc, lvl[0])
                                e_run[0] = acc
                            else:
                                nc.vector.tensor_add(e_run[0], e_run[0], lvl[0])

                    pend = None
                    for kc in range(NKC):
                        s_ps = ps_mm.tile([P, 512], F32, tag="mm", name="s_ps")
                        for i in range(DT):
                            nc.tensor.matmul(
                                s_ps,
                                lhsT=xt_sb[i][:, kc * P:(kc + 1) * P],
                                rhs=u_sb[i][:, qc * 512:(qc + 1) * 512],
                                start=(i == 0),
                                stop=(i == DT - 1),
                            )
                        et = et_pool.tile([P, 512], BF16, tag="et", name="et")
                        nc.scalar.activation(
                            et, s_ps, AF.Exp, bias=gb_sb[:, kc:kc + 1]
                        )
                        if pend is not None:
                            emit_av(*pend)
                        pend = (kc, et)
                    emit_av(*pend)
                    if not last_qc:
                        nc.tensor.matmul(
                            sum_ps,
                            lhsT=ones_col,
                            rhs=e_run[0],
                            start=True,
                            stop=True,
                        )

                    # row sums -> per-partition reciprocals per q-subtile
                    sums_r = small_pool.tile([1, 512], F32R, tag="sums", name="sums")
                    nc.vector.tensor_copy(sums_r, sum_ps)
                    for qs in range(4):
                        # rank-1 bias, pre-scaled by the row sums so the recip
                        # scaling below restores the exact bias; closes the
                        # 32-step accumulation group
                        nc.tensor.matmul(
                            out_ps[qs],
                            lhsT=sums_r[:, qs * P:(qs + 1) * P],
                            rhs=c_row,
                            start=False,
                            stop=True,
                        )
                        r_ps = ps_sum.tile([P, 2], F32, tag="sum", name="r_ps")
                        nc.tensor.matmul(
                            r_ps,
                            lhsT=sums_r[:, qs * P:(qs + 1) * P],
                            rhs=ones_1x2,
                            start=True,
                            stop=True,
                        )
                        rc = small_pool.tile(
                            [P, 1], F32, tag="recip", name="recip", bufs=4
                        )
                        nc.vector.reciprocal(rc, r_ps[:, 0:1])
                        o_sb = out_pool.tile([P, D], F32, tag="outsb", name="outsb")
                        nc.scalar.activation(o_sb, out_ps[qs], AF.Copy, scale=rc)
                        nc.sync.dma_start(
                            out[(qc * 4 + qs) * P:(qc * 4 + qs + 1) * P, :], o_sb
                        )

    nc.compile()
    return nc


_NC_CACHE = None


def _get_nc():
    global _NC_CACHE
    if _NC_CACHE is None:
        _NC_CACHE = build_bass()
    return _NC_CACHE


def make_in_maps(inputs):
    x = np.asarray(inputs["x"], dtype=np.float32)
    w = {k: np.asarray(inputs[k], dtype=np.float64)
         for k in ("Wq", "bq", "Wk", "bk", "Wv", "bv", "Wo", "bo")}

    def tile_rows(m):  # [D, D] -> [P, DT, D] (partition-major d-tiles)
        return np.ascontiguousarray(
            m.reshape(DT, P, D).transpose(1, 0, 2).astype(ml_dtypes.bfloat16)
        )

    a_w = tile_rows(w["Wq"] @ w["Wk"].T)
    b_w = tile_rows(w["Wv"] @ w["Wo"])
    crow = np.ascontiguousarray((w["bv"] @ w["Wo"] + w["bo"]).astype(np.float32))
    wkbq = w["Wk"] @ w["bq"]                     # [D]; scores shift g = x_k . wkbq

    in_maps = []
    for c in range(8):
        b, half = c // 2, c % 2
        own = x[b, half * SQ:(half + 1) * SQ]
        other = x[b, (1 - half) * SQ:(2 - half) * SQ]
        xkv = np.concatenate([own, other], axis=0)           # rolled [S, D] f32
        g = xkv.astype(np.float64) @ wkbq + EXP_SHIFT        # [S]
        gb = np.ascontiguousarray(
            g.astype(np.float32).reshape(NKC, P).T            # [P, NKC]
        )
        in_maps.append({
            "xt": np.ascontiguousarray(xkv.T.astype(ml_dtypes.bfloat16)),
            "a_w": a_w, "b_w": b_w, "crow": crow, "gb": gb,
        })
    return in_maps


def gather_out(results):
    out = np.empty((B, S, D), dtype=np.float32)
    for c in range(8):
        b, half = c // 2, c % 2
        out[b, half * SQ:(half + 1) * SQ] = results[c]["out"]
    return out


def kernel(**inputs):
    nc = _get_nc()
    res = run_bass_kernel_spmd(nc, make_in_maps(inputs), list(range(8)))
    return gather_out(res.results)


if __name__ == "__main__":
    import jax

    import reference

    with jax.default_device(jax.devices("cpu")[0]):
        inp = {k: np.asarray(v) for k, v in reference.setup_inputs().items()}
        expected = np.asarray(reference.reference(**inp))
    actual = kernel(**inp)
    err = np.abs(actual - expected).max()
    rel = np.linalg.norm(actual - expected) / np.linalg.norm(expected)
    print("abs max err", err, "rel err", rel)
